# revision 1
# baseline (speedup 1.0000x reference)
"""HQQLinearLoRA TRN2 kernel: out = x @ W + (x @ A) @ B * 1.0 + bias.

Sharding: data-parallel over tokens (B*S) across 8 NeuronCores; W/bias/lora
replicated. Per core: [M_CORE, D] @ [D, D] with LoRA rank-16 + bias folded
into one extra K=17 accumulation matmul per output tile.

PE runs float32r (rounded fp32, 1 cycle/row). Every PE-instruction input is
last-produced by DVE so each fused-weight-load matmul carries at most one
sync wait (hardware limit on 4-byte-dtype matmuls).
"""
import numpy as np
from contextlib import ExitStack

import concourse.bass as bass
import concourse.tile as tile
import concourse.mybir as mybir
from concourse import bacc
from concourse.bass_utils import run_bass_kernel_spmd
from concourse.masks import make_identity

P = 128
NCORES = 8

# full problem dims (hardcoded per task contract)
B_DIM, S_DIM, D_DIM, R_DIM = 4, 4096, 4096, 16


def build_nc(m_core, d, r, m_blocks, n_tile=512, f32r=True,
             xs_bufs=2, ws_bufs=8, wr_bufs=3, ot_bufs=4, aux_bufs=2,
             dve_transpose=False):
    """One-core program; same program runs SPMD on all cores."""
    KT = d // P
    NT = d // n_tile
    mm_dt = mybir.dt.float32r if f32r else mybir.dt.float32
    f32 = mybir.dt.float32

    nc = bacc.Bacc(target_bir_lowering=False)
    x = nc.declare_dram_parameter("x", [m_core, d], f32, isOutput=False)
    W = nc.declare_dram_parameter("W", [d, d], f32, isOutput=False)
    bias = nc.declare_dram_parameter("bias", [d], f32, isOutput=False)
    lora_A = nc.declare_dram_parameter("lora_A", [d, r], f32, isOutput=False)
    lora_B = nc.declare_dram_parameter("lora_B", [r, d], f32, isOutput=False)
    out = nc.declare_dram_parameter("out", [m_core, d], f32, isOutput=True)

    with tile.TileContext(nc) as tc, ExitStack() as ctx:
        const = ctx.enter_context(tc.tile_pool(name="const", bufs=1))
        xstage = ctx.enter_context(tc.tile_pool(name="xstage", bufs=xs_bufs))
        xtpool = ctx.enter_context(tc.tile_pool(name="xtpool", bufs=1))
        wstage = ctx.enter_context(tc.tile_pool(name="wstage", bufs=ws_bufs))
        wrpool = ctx.enter_context(tc.tile_pool(name="wrpool", bufs=wr_bufs))
        stg = ctx.enter_context(tc.tile_pool(name="stg", bufs=2))
        outstage = ctx.enter_context(tc.tile_pool(name="outstage", bufs=ot_bufs))
        psum_main = ctx.enter_context(
            tc.tile_pool(name="psum_main", bufs=max(m_blocks), space="PSUM"))
        psum_aux = ctx.enter_context(
            tc.tile_pool(name="psum_aux", bufs=aux_bufs, space="PSUM"))

        # identity for PE transpose (fp32 path; HW-validated)
        ident = const.tile([P, P], f32)
        make_identity(nc, ident)

        # lora_A rounded, per k-tile: [P, r]
        a_r = []
        for ki in range(KT):
            ast = stg.tile([P, r], f32, name="ast")
            nc.sync.dma_start(ast[:], lora_A[ki * P:(ki + 1) * P, :])
            ar = const.tile([P, r], mm_dt, name=f"ar{ki}")
            nc.vector.tensor_copy(ar[:], ast[:])
            a_r.append(ar)

        # lora_B and bias rounded (separate tiles: partition bases must be 0)
        b_r = const.tile([r, d], mm_dt)
        bias_r = const.tile([1, d], mm_dt)
        for ni in range(NT):
            sl = slice(ni * n_tile, (ni + 1) * n_tile)
            bst = stg.tile([r, n_tile], f32, name="bst")
            nc.sync.dma_start(bst[:], lora_B[:, sl])
            nc.vector.tensor_copy(b_r[:, sl], bst[:])
            bist = stg.tile([1, n_tile], f32, name="bist")
            nc.sync.dma_start(bist[:], bias[sl].unsqueeze(0))
            nc.vector.tensor_copy(bias_r[:, sl], bist[:])

        # P1T = (x@A)^T: [r, m_core]; ones row for bias outer product
        p1t = const.tile([r, m_core], mm_dt)
        ones_st = const.tile([1, m_core], f32)
        nc.vector.memset(ones_st[:], 1.0)
        ones_r = const.tile([1, m_core], mm_dt)
        nc.vector.tensor_copy(ones_r[:], ones_st[:])

        mb_max = max(m_blocks)
        mt0 = 0  # running m-tile offset
        for mb in m_blocks:
            xtb = xtpool.tile([P, KT, mb_max * P], mm_dt, name="xtblock")
            # ---- transpose phase: x[mt*P:(mt+1)*P, :] -> xtb[:, ki, mi*P:]
            for mi in range(mb):
                mt = mt0 + mi
                xs = xstage.tile([P, d], f32, name="xs")
                nc.gpsimd.dma_start(xs[:], x[mt * P:(mt + 1) * P, :])
                if dve_transpose:
                    # 32x32-block DVE transpose straight into xtb (f32r out).
                    # Block row j of the output comes from partition strip j
                    # of the input with free offsets swapped.
                    for ki in range(KT):
                        for j in range(P // 32):
                            nc.vector.transpose(
                                xtb[:, ki, mi * P:(mi + 1) * P].rearrange(
                                    "p (b f) -> p b f", f=32)[32 * j:32 * (j + 1)]
                                .transpose(0, 1),
                                xs[:, ki * P + 32 * j: ki * P + 32 * (j + 1)]
                                .rearrange("(b q) f -> q b f", q=32),
                            )
                else:
                    for ki in range(KT):
                        pst = psum_aux.tile([P, P], f32, name="aux")
                        nc.tensor.transpose(pst[:], xs[:, ki * P:(ki + 1) * P],
                                            ident[:])
                        # copyback rounds to f32r for the main matmuls
                        nc.vector.tensor_copy(xtb[:, ki, mi * P:(mi + 1) * P],
                                              pst[:])
            # ---- P1T chunks for this block (free dim 256..512 per chunk)
            done = 0
            while done < mb:
                cn = min(4, mb - done)
                cs = cn * P
                psl = psum_aux.tile([r, 512], f32, name="aux")
                for ki in range(KT):
                    nc.tensor.matmul(
                        psl[:, :cs], a_r[ki][:],
                        xtb[:, ki, done * P:done * P + cs],
                        start=(ki == 0), stop=(ki == KT - 1))
                nc.vector.tensor_copy(
                    p1t[:, (mt0 + done) * P:(mt0 + done) * P + cs], psl[:, :cs])
                done += cn
            # ---- GEMM phase: stream W once per block
            for ni in range(NT):
                nsl = slice(ni * n_tile, (ni + 1) * n_tile)
                pss = [psum_main.tile([P, n_tile], f32, name="mm") for _ in range(mb)]
                for ki in range(KT):
                    # W goes on HWDGE with wstage bufs=8: slot reuse distance
                    # is a multiple of the 8 HWDGE sem lanes, so the WAW dep
                    # is same-lane (FIFO) and the DMA carries only the DVE
                    # recycle wait.
                    ws = wstage.tile([P, n_tile], f32, name="ws")
                    nc.sync.dma_start(ws[:], W[ki * P:(ki + 1) * P, nsl])
                    wr = wrpool.tile([P, n_tile], mm_dt, name="wr")
                    nc.vector.tensor_copy(wr[:], ws[:])
                    for mi in range(mb):
                        nc.tensor.matmul(
                            pss[mi][:], xtb[:, ki, mi * P:(mi + 1) * P],
                            wr[:], start=(ki == 0), stop=False)
                for mi in range(mb):
                    mt = mt0 + mi
                    nc.tensor.matmul(
                        pss[mi][:], p1t[:, mt * P:(mt + 1) * P], b_r[:, nsl],
                        start=False, stop=False)
                    nc.tensor.matmul(
                        pss[mi][:], ones_r[:, mt * P:(mt + 1) * P], bias_r[:, nsl],
                        start=False, stop=True)
                    ot = outstage.tile([P, n_tile], f32, name="ot")
                    nc.vector.tensor_copy(ot[:], pss[mi][:])
                    nc.gpsimd.dma_start(out[mt * P:(mt + 1) * P, nsl], ot[:])
            mt0 += mb
    nc.compile()
    return nc


_CACHE = {}


def _get_nc(key, *args, **kw):
    if key not in _CACHE:
        _CACHE[key] = build_nc(*args, **kw)
    return _CACHE[key]


def kernel(x, W, bias, lora_A, lora_B, _trace=False):
    Bb, S, D = x.shape
    R = lora_A.shape[1]
    M = Bb * S
    m_core = M // NCORES
    m_blocks = [4, 4, 4, 4]
    nc = _get_nc(("full", m_core, D, R), m_core, D, R, m_blocks)

    xf = np.ascontiguousarray(x.reshape(M, D), dtype=np.float32)
    W = np.ascontiguousarray(W, dtype=np.float32)
    bias = np.ascontiguousarray(bias, dtype=np.float32)
    lora_A = np.ascontiguousarray(lora_A, dtype=np.float32)
    lora_B = np.ascontiguousarray(lora_B, dtype=np.float32)

    in_maps = []
    for c in range(NCORES):
        in_maps.append({
            "x": xf[c * m_core:(c + 1) * m_core],
            "W": W, "bias": bias, "lora_A": lora_A, "lora_B": lora_B,
        })
    res = run_bass_kernel_spmd(nc, in_maps, list(range(NCORES)), trace=_trace)
    outs = [res.results[c]["out"] for c in range(NCORES)]
    full = np.concatenate(outs, axis=0).reshape(Bb, S, D).astype(x.dtype)
    if _trace:
        return full, res
    return full



# revision 6
# speedup vs baseline: 1.0962x; 1.0962x over previous
"""HQQLinearLoRA TRN2 kernel: out = x @ W + (x @ A) @ B * 1.0 + bias.

Sharding: data-parallel over tokens (B*S) across 8 NeuronCores; W/bias/lora
replicated. Per core: [M_CORE, D] @ [D, D] with LoRA rank-16 + bias folded
into one extra K=17 accumulation matmul per output tile.

PE runs float32r (1 cycle/row when the moving dim >= 256). Every f32r
matmul operand must be last-produced by a rounding instruction (BIR
verifier rule), so W tiles are DMA'd as f32 and rounded by DVE copies;
x is transposed on PE in f32 and the PSUM->SBUF copyback rounds to f32r.

Structure (PE is the bottleneck; everything else is shaped to keep it fed):
- m_blocks=[8,8]: W is streamed twice (201MB total DMA/core vs 335MB at
  mb=4).
- The x-transpose runs fused inside each block's FIRST n-tile pass (no
  standalone transpose phase => no PE bubble at block boundaries). That
  pass is split into two half-passes of 4 m-tiles so PSUM fits:
  4 main accumulators + 1 p1t (LoRA xT@A) + 3 rotating transpose banks.
- Transposes run one k-tile ahead of the matmuls that consume them so the
  DVE copyback hides under the previous k-tile's matmuls.
- LoRA + bias are one K=17 matmul: p1t row 16 is ones (pre-set via a
  staged f32 memset+rounding copy; engines can't write at partition base
  16, so whole-tile base-0 ops only), b17 = [lora_B; bias] staged the
  same way.
- Queues: W tiles on SP HWDGE, x chunks on gpsimd SWDGE, out stores on
  Activation HWDGE.
"""
import numpy as np
from contextlib import ExitStack

import concourse.bass as bass
import concourse.tile as tile
import concourse.mybir as mybir
from concourse import bacc
from concourse.bass_utils import run_bass_kernel_spmd
from concourse.masks import make_identity

P = 128
NCORES = 8

# full problem dims (hardcoded per task contract)
B_DIM, S_DIM, D_DIM, R_DIM = 4, 4096, 4096, 16


def build_nc(m_core, d, r, m_blocks, n_tile=512, kg=4,
             xs_bufs=8, ws_bufs=6, wr_bufs=3, ot_bufs=4):
    """One-core program; same program runs SPMD on all cores."""
    KT = d // P
    NT = d // n_tile
    R17 = r + 1
    f32 = mybir.dt.float32
    f32r = mybir.dt.float32r

    nc = bacc.Bacc(target_bir_lowering=False)
    x = nc.declare_dram_parameter("x", [m_core, d], f32, isOutput=False)
    W = nc.declare_dram_parameter("W", [d, d], f32, isOutput=False)
    bias = nc.declare_dram_parameter("bias", [d], f32, isOutput=False)
    lora_A = nc.declare_dram_parameter("lora_A", [d, r], f32, isOutput=False)
    lora_B = nc.declare_dram_parameter("lora_B", [r, d], f32, isOutput=False)
    out = nc.declare_dram_parameter("out", [m_core, d], f32, isOutput=True)

    with tile.TileContext(nc) as tc, ExitStack() as ctx:
        const = ctx.enter_context(tc.tile_pool(name="const", bufs=1))
        stg = ctx.enter_context(tc.tile_pool(name="stg", bufs=1))
        xsp = ctx.enter_context(tc.tile_pool(name="xsp", bufs=xs_bufs))
        wstage = ctx.enter_context(tc.tile_pool(name="wstage", bufs=ws_bufs))
        wrpool = ctx.enter_context(tc.tile_pool(name="wrpool", bufs=wr_bufs))
        otp = ctx.enter_context(tc.tile_pool(name="otp", bufs=ot_bufs))
        xtbp = ctx.enter_context(tc.tile_pool(name="xtbp", bufs=1))
        ps_main = ctx.enter_context(
            tc.tile_pool(name="ps_main", bufs=4, space="PSUM"))
        ps_p1t = ctx.enter_context(
            tc.tile_pool(name="ps_p1t", bufs=1, space="PSUM"))
        ps_aux = ctx.enter_context(
            tc.tile_pool(name="ps_aux", bufs=3, space="PSUM"))

        # identity for PE transpose (f32 path)
        ident = const.tile([P, P], f32)
        make_identity(nc, ident)

        # lora_A as [P, KT, r] f32r (one DMA; partition p holds rows ki*P+p)
        a_st = stg.tile([P, KT, r], f32, name="stg")
        nc.sync.dma_start(
            a_st[:], lora_A.rearrange("(k p) r -> p k r", p=P))
        a_r = const.tile([P, KT, r], f32r)
        nc.vector.tensor_copy(a_r[:], a_st[:])

        # b17 = [lora_B; bias]: [R17, d] f32r, staged+rounded in halves
        b17 = const.tile([R17, d], f32r)
        for h in range(2):
            hd = d // 2
            hsl = slice(h * hd, (h + 1) * hd)
            bst = stg.tile([R17, hd], f32, name="stg")
            nc.sync.dma_start(bst[0:r, :], lora_B[:, hsl])
            nc.sync.dma_start(bst[r:R17, :], bias[hsl].unsqueeze(0))
            nc.vector.tensor_copy(b17[:, hsl], bst[:])

        # p1t = [(x@A)^T; ones]: [R17, m_core] f32r. Pre-fill ALL rows with
        # rounded 1.0 (row 16 keeps it; rows 0..16 get overwritten per
        # half-pass below).
        p1t = const.tile([R17, m_core], f32r)
        p1st = stg.tile([R17, m_core], f32, name="stg")
        nc.vector.memset(p1st[:], 1.0)
        nc.vector.tensor_copy(p1t[:], p1st[:])

        mt0 = 0
        for mb in m_blocks:
            xtb = xtbp.tile([P, KT, mb * P], f32r, name="xtb")
            nhp = mb // 4
            for hp in range(nhp):
                # ---- fused first n-tile pass (ni=0): transpose + GEMM
                mis = [hp * 4 + i for i in range(4)]
                nsl = slice(0, n_tile)
                pss = [ps_main.tile([P, n_tile], f32, name="mm") for _ in mis]
                pp1 = ps_p1t.tile([r, n_tile], f32, name="mm")
                xs_tiles = {}

                def load_group(g):
                    for mi in mis:
                        mt = mt0 + mi
                        xst = xsp.tile([P, kg * P], f32, name="xs")
                        nc.gpsimd.dma_start(
                            xst[:],
                            x[mt * P:(mt + 1) * P, g * kg * P:(g + 1) * kg * P])
                        xs_tiles[(mi, g)] = xst

                def transpose_ki(ki):
                    g, lk = ki // kg, ki % kg
                    for mi in mis:
                        pst = ps_aux.tile([P, P], f32, name="mm")
                        nc.tensor.transpose(
                            pst[:],
                            xs_tiles[(mi, g)][:, lk * P:(lk + 1) * P],
                            ident[:])
                        # copyback rounds to f32r for the main matmuls
                        nc.vector.tensor_copy(
                            xtb[:, ki, mi * P:(mi + 1) * P], pst[:])

                load_group(0)
                transpose_ki(0)
                for ki in range(KT):
                    if ki % kg == 0 and ki + kg < KT:
                        load_group(ki // kg + 1)
                    if ki + 1 < KT:
                        transpose_ki(ki + 1)
                    ws = wstage.tile([P, n_tile], f32, name="ws")
                    nc.sync.dma_start(ws[:], W[ki * P:(ki + 1) * P, nsl])
                    wr = wrpool.tile([P, n_tile], f32r, name="wr")
                    nc.vector.tensor_copy(wr[:], ws[:])
                    for j, mi in enumerate(mis):
                        nc.tensor.matmul(
                            pss[j][:], xtb[:, ki, mi * P:(mi + 1) * P],
                            wr[:], start=(ki == 0), stop=False)
                    nc.tensor.matmul(
                        pp1[:], a_r[:, ki, :],
                        xtb[:, ki, hp * n_tile:(hp + 1) * n_tile],
                        start=(ki == 0), stop=(ki == KT - 1))
                # p1t chunk for this half-pass (rounds f32 -> f32r)
                nc.vector.tensor_copy(
                    p1t[0:r, (mt0 + hp * 4) * P:(mt0 + hp * 4) * P + n_tile],
                    pp1[:])
                for j, mi in enumerate(mis):
                    mt = mt0 + mi
                    nc.tensor.matmul(
                        pss[j][:], p1t[:, mt * P:(mt + 1) * P], b17[:, nsl],
                        start=False, stop=True)
                    ot = otp.tile([P, n_tile], f32, name="ot")
                    nc.vector.tensor_copy(ot[:], pss[j][:])
                    nc.scalar.dma_start(out[mt * P:(mt + 1) * P, nsl], ot[:])

            # ---- remaining n-tiles: plain GEMM with 8-wide PSUM
            for ni in range(1, NT):
                nsl = slice(ni * n_tile, (ni + 1) * n_tile)
                pss = []
                for mi in range(mb):
                    pool = (ps_main if mi < 4 else
                            ps_aux if mi < 7 else ps_p1t)
                    pss.append(pool.tile([P, n_tile], f32, name="mm"))
                for ki in range(KT):
                    ws = wstage.tile([P, n_tile], f32, name="ws")
                    nc.sync.dma_start(ws[:], W[ki * P:(ki + 1) * P, nsl])
                    wr = wrpool.tile([P, n_tile], f32r, name="wr")
                    nc.vector.tensor_copy(wr[:], ws[:])
                    for mi in range(mb):
                        nc.tensor.matmul(
                            pss[mi][:], xtb[:, ki, mi * P:(mi + 1) * P],
                            wr[:], start=(ki == 0), stop=False)
                for mi in range(mb):
                    mt = mt0 + mi
                    nc.tensor.matmul(
                        pss[mi][:], p1t[:, mt * P:(mt + 1) * P], b17[:, nsl],
                        start=False, stop=True)
                    ot = otp.tile([P, n_tile], f32, name="ot")
                    nc.vector.tensor_copy(ot[:], pss[mi][:])
                    nc.scalar.dma_start(out[mt * P:(mt + 1) * P, nsl], ot[:])
            mt0 += mb
    nc.compile()
    return nc


_CACHE = {}


def _get_nc(key, *args, **kw):
    if key not in _CACHE:
        _CACHE[key] = build_nc(*args, **kw)
    return _CACHE[key]


def kernel(x, W, bias, lora_A, lora_B, _trace=False):
    Bb, S, D = x.shape
    R = lora_A.shape[1]
    M = Bb * S
    m_core = M // NCORES
    m_blocks = [8, 8]
    nc = _get_nc(("full", m_core, D, R), m_core, D, R, m_blocks)

    xf = np.ascontiguousarray(x.reshape(M, D), dtype=np.float32)
    W = np.ascontiguousarray(W, dtype=np.float32)
    bias = np.ascontiguousarray(bias, dtype=np.float32)
    lora_A = np.ascontiguousarray(lora_A, dtype=np.float32)
    lora_B = np.ascontiguousarray(lora_B, dtype=np.float32)

    in_maps = []
    for c in range(NCORES):
        in_maps.append({
            "x": xf[c * m_core:(c + 1) * m_core],
            "W": W, "bias": bias, "lora_A": lora_A, "lora_B": lora_B,
        })
    res = run_bass_kernel_spmd(nc, in_maps, list(range(NCORES)), trace=_trace)
    outs = [res.results[c]["out"] for c in range(NCORES)]
    full = np.concatenate(outs, axis=0).reshape(Bb, S, D).astype(x.dtype)
    if _trace:
        return full, res
    return full


# revision 17
# speedup vs baseline: 1.1286x; 1.0295x over previous
"""HQQLinearLoRA TRN2 kernel: out = x @ W + (x @ A) @ B * 1.0 + bias.

Sharding: data-parallel over tokens (B*S) across 8 NeuronCores; W/bias/lora
replicated. Per core: [M_CORE, D] @ [D, D] with LoRA rank-16 + bias folded
into one extra K=17 accumulation matmul per output tile.

PE runs float32r (1 cycle/row when the moving dim >= 256). Every f32r
matmul operand must be last-produced by a rounding instruction (BIR
verifier rule), so W tiles are DMA'd as f32 and rounded by DVE copies;
x is transposed on PE in f32 and the PSUM->SBUF copyback rounds to f32r.

Structure (PE is the bottleneck; everything else is shaped to keep it fed):
- m_blocks=[8,8]: W is streamed twice (201MB total DMA/core vs 335MB at
  mb=4).
- The x-transpose runs fused inside each block's FIRST n-tile pass (no
  standalone transpose phase => no PE bubble at block boundaries). That
  pass is split into two half-passes of 4 m-tiles so PSUM fits:
  4 main accumulators + 1 p1t (LoRA xT@A) + 3 rotating transpose banks.
- Transposes run one k-tile ahead of the matmuls that consume them so the
  DVE copyback hides under the previous k-tile's matmuls.
- LoRA + bias are one K=17 matmul: p1t row 16 is ones (whole tile pre-
  filled with rounded 1.0 since engines can't write at partition base 16),
  b17 = [lora_B; bias] staged f32 then rounded.
- The last k-iteration interleaves [matmul, lora+stop] per m-tile so PSUM
  drains start ~1.7us earlier and the next n-tile never waits on a drain.
- Constant staging copies are spread across the first k-loop so the DVE
  queue is never busy with them when PE needs a wr tile or copyback.
- Queues: W tiles on SP HWDGE, x chunks on gpsimd SWDGE, out stores on
  Activation HWDGE.
"""
import numpy as np
from contextlib import ExitStack

import concourse.bass as bass
import concourse.tile as tile
import concourse.mybir as mybir
from concourse import bacc
from concourse.bass_utils import run_bass_kernel_spmd
from concourse.masks import make_identity

P = 128
NCORES = 8

# full problem dims (hardcoded per task contract)
B_DIM, S_DIM, D_DIM, R_DIM = 4, 4096, 4096, 16


def build_nc(m_core, d, r, m_blocks, n_tile=512, kg=4,
             xs_bufs=8, ws_bufs=5, wr_bufs=3, ot_bufs=8):
    """One-core program; same program runs SPMD on all cores."""
    KT = d // P
    NT = d // n_tile
    R17 = r + 1
    f32 = mybir.dt.float32
    f32r = mybir.dt.float32r

    nc = bacc.Bacc(target_bir_lowering=False)
    x = nc.declare_dram_parameter("x", [m_core, d], f32, isOutput=False)
    W = nc.declare_dram_parameter("W", [d, d], f32, isOutput=False)
    bias = nc.declare_dram_parameter("bias", [d], f32, isOutput=False)
    lora_A = nc.declare_dram_parameter("lora_A", [d, r], f32, isOutput=False)
    lora_B = nc.declare_dram_parameter("lora_B", [r, d], f32, isOutput=False)
    out = nc.declare_dram_parameter("out", [m_core, d], f32, isOutput=True)

    with tile.TileContext(nc) as tc, ExitStack() as ctx:
        const = ctx.enter_context(tc.tile_pool(name="const", bufs=1))
        stg = ctx.enter_context(tc.tile_pool(name="stg", bufs=1))
        xsp = ctx.enter_context(tc.tile_pool(name="xsp", bufs=xs_bufs))
        wstage = ctx.enter_context(tc.tile_pool(name="wstage", bufs=ws_bufs))
        wrpool = ctx.enter_context(tc.tile_pool(name="wrpool", bufs=wr_bufs))
        otp = ctx.enter_context(tc.tile_pool(name="otp", bufs=ot_bufs))
        xtbp = ctx.enter_context(tc.tile_pool(name="xtbp", bufs=1))
        ps_main = ctx.enter_context(
            tc.tile_pool(name="ps_main", bufs=4, space="PSUM"))
        ps_p1t = ctx.enter_context(
            tc.tile_pool(name="ps_p1t", bufs=1, space="PSUM"))
        ps_aux = ctx.enter_context(
            tc.tile_pool(name="ps_aux", bufs=3, space="PSUM"))

        # identity for PE transpose (f32 path)
        ident = const.tile([P, P], f32)
        make_identity(nc, ident)

        # lora_A as [P, KT, r] f32r (one DMA; partition p holds rows ki*P+p)
        a_st = stg.tile([P, KT, r], f32, name="stg")
        nc.scalar.dma_start(
            a_st[:], lora_A.rearrange("(k p) r -> p k r", p=P))
        a_r = const.tile([P, KT, r], f32r)
        # (a_r rounding copy is emitted inside the first k-loop)

        b17 = const.tile([R17, d], f32r)
        p1t = const.tile([R17, m_core], f32r)

        def stage_consts(step):
            # Emitted mid-k-loop of the first half-pass: keeps these DVE
            # copies out of the startup critical path.
            if step < 4:
                # b17 = [lora_B; bias]: [R17, d] f32r, staged in quarters
                qd = d // 4
                hsl = slice(step * qd, (step + 1) * qd)
                bst = stg.tile([R17, qd], f32, name="stg")
                nc.scalar.dma_start(bst[0:r, :], lora_B[:, hsl])
                nc.scalar.dma_start(bst[r:R17, :], bias[hsl].unsqueeze(0))
                nc.vector.tensor_copy(b17[:, hsl], bst[:])
            else:
                # p1t pre-fill with rounded 1.0: row 16 keeps it (ones for
                # the bias outer product); rows 0..16 get overwritten by
                # the per-half-pass chunk copies. Staged in halves.
                hm = m_core // 2
                hsl = slice((step - 4) * hm, (step - 3) * hm)
                p1st = stg.tile([R17, hm], f32, name="stg")
                nc.gpsimd.memset(p1st[:], 1.0)
                nc.vector.tensor_copy(p1t[:, hsl], p1st[:])

        # W tiles are emitted one consumer-step ahead (DMA + DVE rounding
        # copy), so at n-tile/block boundaries the next wr is already
        # rounded before the PSUM drain copies queue up on DVE.
        wsteps = []
        for mb in m_blocks:
            for hp in range(mb // 4):
                wsteps += [(0, ki) for ki in range(KT)]
            for ni in range(1, NT):
                wsteps += [(ni, ki) for ki in range(KT)]
        wq = []
        widx = [0]

        def emit_next_w():
            if widx[0] < len(wsteps):
                ni, ki = wsteps[widx[0]]
                widx[0] += 1
                nsl = slice(ni * n_tile, (ni + 1) * n_tile)
                ws = wstage.tile([P, n_tile], f32, name="ws")
                nc.sync.dma_start(ws[:], W[ki * P:(ki + 1) * P, nsl])
                wr = wrpool.tile([P, n_tile], f32r, name="wr")
                nc.vector.tensor_copy(wr[:], ws[:])
                wq.append(wr)

        emit_next_w()

        mt0 = 0
        first = True
        for mb in m_blocks:
            xtb = xtbp.tile([P, KT, mb * P], f32r, name="xtb")
            nhp = mb // 4
            for hp in range(nhp):
                # ---- fused first n-tile pass (ni=0): transpose + GEMM
                mis = [hp * 4 + i for i in range(4)]
                nsl = slice(0, n_tile)
                pss = [ps_main.tile([P, n_tile], f32, name="mm") for _ in mis]
                pp1 = ps_p1t.tile([r, n_tile], f32, name="mm")
                xs_tiles = {}

                def load_group(g):
                    for mi in mis:
                        mt = mt0 + mi
                        xst = xsp.tile([P, kg * P], f32, name="xs")
                        nc.gpsimd.dma_start(
                            xst[:],
                            x[mt * P:(mt + 1) * P, g * kg * P:(g + 1) * kg * P])
                        xs_tiles[(mi, g)] = xst

                def transpose_ki(ki):
                    g, lk = ki // kg, ki % kg
                    for mi in mis:
                        pst = ps_aux.tile([P, P], f32, name="mm")
                        nc.tensor.transpose(
                            pst[:],
                            xs_tiles[(mi, g)][:, lk * P:(lk + 1) * P],
                            ident[:])
                        # copyback rounds to f32r for the main matmuls
                        nc.vector.tensor_copy(
                            xtb[:, ki, mi * P:(mi + 1) * P], pst[:])

                out_dmas = []

                def drain(j, mi):
                    # lora+bias accumulation, then PSUM -> SBUF (alternating
                    # DVE/Act so the boundary drain burst fits both queues);
                    # the DRAM stores are batched after the copies so they
                    # can't delay an Act copy in queue order.
                    mt = mt0 + mi
                    nc.tensor.matmul(
                        pss[j][:], p1t[:, mt * P:(mt + 1) * P], b17[:, nsl],
                        start=False, stop=True)
                    ot = otp.tile([P, n_tile], f32, name="ot")
                    if mi % 4 == 3:
                        nc.scalar.copy(ot[:], pss[j][:])
                    else:
                        nc.vector.tensor_copy(ot[:], pss[j][:])
                    out_dmas.append((out[mt * P:(mt + 1) * P, nsl], ot))

                def flush_out_dmas():
                    for dst, ot in out_dmas:
                        nc.scalar.dma_start(dst, ot[:])
                    out_dmas.clear()

                def pp1_mm(ki):
                    nc.tensor.matmul(
                        pp1[:], a_r[:, ki, :],
                        xtb[:, ki, hp * n_tile:(hp + 1) * n_tile],
                        start=(ki == 0), stop=(ki == KT - 1))

                load_group(0)
                transpose_ki(0)
                for ki in range(KT):
                    wr = wq.pop(0)
                    emit_next_w()
                    if ki % kg == 0 and ki + kg < KT:
                        load_group(ki // kg + 1)
                    if ki + 1 < KT:
                        transpose_ki(ki + 1)
                    last = ki == KT - 1
                    for j, mi in enumerate(mis):
                        nc.tensor.matmul(
                            pss[j][:], xtb[:, ki, mi * P:(mi + 1) * P],
                            wr[:], start=(ki == 0), stop=False)
                        if last:
                            drain(j, mi)
                    if first and ki == 0:
                        # a_r rounding copy off the startup critical path
                        # (first needed by pp1_mm(0) below)
                        nc.vector.tensor_copy(a_r[:], a_st[:])
                    # pp1 runs one k-tile behind the transposes but one AHEAD
                    # of this loop for ki>=1, so its stop lands in iteration
                    # KT-2 and the p1t rounding copy hides under the last
                    # main matmuls.
                    if ki == 0:
                        pp1_mm(0)
                        pp1_mm(1)
                    elif ki < KT - 1:
                        pp1_mm(ki + 1)
                        if ki == KT - 2:
                            nc.vector.tensor_copy(
                                p1t[0:r, (mt0 + hp * 4) * P:
                                    (mt0 + hp * 4) * P + n_tile],
                                pp1[:])
                    if first and 4 <= ki < 28 and ki % 4 == 0:
                        stage_consts(ki // 4 - 1)
                flush_out_dmas()
                first = False

            # ---- remaining n-tiles: plain GEMM with 8-wide PSUM
            for ni in range(1, NT):
                nsl = slice(ni * n_tile, (ni + 1) * n_tile)
                pss = []
                for mi in range(mb):
                    pool = (ps_main if mi < 4 else
                            ps_aux if mi < 7 else ps_p1t)
                    pss.append(pool.tile([P, n_tile], f32, name="mm"))
                # at the last n-tile of a block, drain the aux/p1t banks
                # first: the next block's transposes/pp1 need those slots
                # before the main banks
                mi_order = (list(range(4, mb)) + list(range(4))
                            if ni == NT - 1 else list(range(mb)))
                out_dmas = []
                for ki in range(KT):
                    wr = wq.pop(0)
                    emit_next_w()
                    last = ki == KT - 1
                    for mi in (mi_order if last else range(mb)):
                        mt = mt0 + mi
                        nc.tensor.matmul(
                            pss[mi][:], xtb[:, ki, mi * P:(mi + 1) * P],
                            wr[:], start=(ki == 0), stop=False)
                        if last:
                            nc.tensor.matmul(
                                pss[mi][:], p1t[:, mt * P:(mt + 1) * P],
                                b17[:, nsl], start=False, stop=True)
                            ot = otp.tile([P, n_tile], f32, name="ot")
                            if mi % 4 == 3:
                                nc.scalar.copy(ot[:], pss[mi][:])
                            else:
                                nc.vector.tensor_copy(ot[:], pss[mi][:])
                            out_dmas.append(
                                (out[mt * P:(mt + 1) * P, nsl], ot))
                for dst, ot in out_dmas:
                    nc.scalar.dma_start(dst, ot[:])
            mt0 += mb
    nc.compile()
    return nc


_CACHE = {}


def _get_nc(key, *args, **kw):
    if key not in _CACHE:
        _CACHE[key] = build_nc(*args, **kw)
    return _CACHE[key]


def kernel(x, W, bias, lora_A, lora_B, _trace=False):
    Bb, S, D = x.shape
    R = lora_A.shape[1]
    M = Bb * S
    m_core = M // NCORES
    m_blocks = [8, 8]
    nc = _get_nc(("full", m_core, D, R), m_core, D, R, m_blocks)

    xf = np.ascontiguousarray(x.reshape(M, D), dtype=np.float32)
    W = np.ascontiguousarray(W, dtype=np.float32)
    bias = np.ascontiguousarray(bias, dtype=np.float32)
    lora_A = np.ascontiguousarray(lora_A, dtype=np.float32)
    lora_B = np.ascontiguousarray(lora_B, dtype=np.float32)

    in_maps = []
    for c in range(NCORES):
        in_maps.append({
            "x": xf[c * m_core:(c + 1) * m_core],
            "W": W, "bias": bias, "lora_A": lora_A, "lora_B": lora_B,
        })
    res = run_bass_kernel_spmd(nc, in_maps, list(range(NCORES)), trace=_trace)
    outs = [res.results[c]["out"] for c in range(NCORES)]
    full = np.concatenate(outs, axis=0).reshape(Bb, S, D).astype(x.dtype)
    if _trace:
        return full, res
    return full


# revision 24
# speedup vs baseline: 1.1293x; 1.0006x over previous
"""HQQLinearLoRA TRN2 kernel: out = x @ W + (x @ A) @ B * 1.0 + bias.

Sharding: data-parallel over tokens (B*S) across 8 NeuronCores; W/bias/lora
replicated. Per core: [M_CORE, D] @ [D, D] with LoRA rank-16 + bias folded
into one extra K=17 accumulation matmul per output tile.

PE runs float32r (1 cycle/row when the moving dim >= 256). Every f32r
matmul operand must be last-produced by a rounding instruction (BIR
verifier rule), so W tiles are DMA'd as f32 and rounded by DVE copies;
x is transposed on PE in f32 and the PSUM->SBUF copyback rounds to f32r.

Structure (PE is the bottleneck; everything else is shaped to keep it fed):
- m_blocks=[8,8]: W is streamed twice (201MB total DMA/core vs 335MB at
  mb=4).
- The x-transpose runs fused inside each block's FIRST n-tile pass (no
  standalone transpose phase => no PE bubble at block boundaries). That
  pass is split into two half-passes of 4 m-tiles so PSUM fits:
  4 main accumulators + 1 p1t (LoRA xT@A) + 3 rotating transpose banks.
- Transposes run one k-tile ahead of the matmuls that consume them so the
  DVE copyback hides under the previous k-tile's matmuls.
- LoRA + bias are one K=17 matmul: p1t row 16 is ones (whole tile pre-
  filled with rounded 1.0 since engines can't write at partition base 16),
  b17 = [lora_B; bias] staged f32 then rounded.
- The last k-iteration interleaves [matmul, lora+stop] per m-tile so PSUM
  drains start ~1.7us earlier and the next n-tile never waits on a drain.
- Constant staging copies are spread across the first k-loop so the DVE
  queue is never busy with them when PE needs a wr tile or copyback.
- Queues: W tiles on SP HWDGE, x chunks on gpsimd SWDGE, out stores on
  Activation HWDGE.
"""
import numpy as np
from contextlib import ExitStack

import concourse.bass as bass
import concourse.tile as tile
import concourse.mybir as mybir
from concourse import bacc
from concourse.bass_utils import run_bass_kernel_spmd
from concourse.masks import make_identity

P = 128
NCORES = 8

# full problem dims (hardcoded per task contract)
B_DIM, S_DIM, D_DIM, R_DIM = 4, 4096, 4096, 16


def build_nc(m_core, d, r, m_blocks, n_tile=512, kg=4,
             xs_bufs=8, ws_bufs=5, wr_bufs=3, ot_bufs=4):
    """One-core program; same program runs SPMD on all cores."""
    KT = d // P
    NT = d // n_tile
    R17 = r + 1
    f32 = mybir.dt.float32
    f32r = mybir.dt.float32r

    nc = bacc.Bacc(target_bir_lowering=False)
    x = nc.declare_dram_parameter("x", [m_core, d], f32, isOutput=False)
    W = nc.declare_dram_parameter("W", [d, d], f32, isOutput=False)
    bias = nc.declare_dram_parameter("bias", [d], f32, isOutput=False)
    lora_A = nc.declare_dram_parameter("lora_A", [d, r], f32, isOutput=False)
    lora_B = nc.declare_dram_parameter("lora_B", [r, d], f32, isOutput=False)
    out = nc.declare_dram_parameter("out", [m_core, d], f32, isOutput=True)

    with tile.TileContext(nc) as tc, ExitStack() as ctx:
        const = ctx.enter_context(tc.tile_pool(name="const", bufs=1))
        stg = ctx.enter_context(tc.tile_pool(name="stg", bufs=1))
        xsp = ctx.enter_context(tc.tile_pool(name="xsp", bufs=xs_bufs))
        wstage = ctx.enter_context(tc.tile_pool(name="wstage", bufs=ws_bufs))
        wrpool = ctx.enter_context(tc.tile_pool(name="wrpool", bufs=wr_bufs))
        otp = ctx.enter_context(tc.tile_pool(name="otp", bufs=ot_bufs))
        xtbp = ctx.enter_context(tc.tile_pool(name="xtbp", bufs=1))
        ps_main = ctx.enter_context(
            tc.tile_pool(name="ps_main", bufs=4, space="PSUM"))
        ps_p1t = ctx.enter_context(
            tc.tile_pool(name="ps_p1t", bufs=1, space="PSUM"))
        ps_aux = ctx.enter_context(
            tc.tile_pool(name="ps_aux", bufs=3, space="PSUM"))

        # identity for PE transpose (f32 path)
        ident = const.tile([P, P], f32)
        make_identity(nc, ident)

        # lora_A as [P, KT, r] f32r (one DMA; partition p holds rows ki*P+p)
        a_st = stg.tile([P, KT, r], f32, name="stg")
        nc.scalar.dma_start(
            a_st[:], lora_A.rearrange("(k p) r -> p k r", p=P))
        a_r = const.tile([P, KT, r], f32r)
        # (a_r rounding copy is emitted inside the first k-loop)

        b17 = const.tile([R17, d], f32r)
        p1t = const.tile([R17, m_core], f32r)

        def stage_consts(step):
            # Emitted mid-k-loop of the first half-pass: keeps these DVE
            # copies out of the startup critical path.
            if step < 4:
                # b17 = [lora_B; bias]: [R17, d] f32r, staged in quarters
                qd = d // 4
                hsl = slice(step * qd, (step + 1) * qd)
                bst = stg.tile([R17, qd], f32, name="stg")
                nc.scalar.dma_start(bst[0:r, :], lora_B[:, hsl])
                nc.scalar.dma_start(bst[r:R17, :], bias[hsl].unsqueeze(0))
                nc.vector.tensor_copy(b17[:, hsl], bst[:])
            else:
                # p1t pre-fill with rounded 1.0: row 16 keeps it (ones for
                # the bias outer product); rows 0..16 get overwritten by
                # the per-half-pass chunk copies. Staged in halves.
                hm = m_core // 2
                hsl = slice((step - 4) * hm, (step - 3) * hm)
                p1st = stg.tile([R17, hm], f32, name="stg")
                nc.gpsimd.memset(p1st[:], 1.0)
                nc.vector.tensor_copy(p1t[:, hsl], p1st[:])

        # W tiles are emitted one consumer-step ahead (DMA + DVE rounding
        # copy), so at n-tile/block boundaries the next wr is already
        # rounded before the PSUM drain copies queue up on DVE.
        wsteps = []
        for mb in m_blocks:
            for hp in range(mb // 4):
                wsteps += [(0, ki) for ki in range(KT)]
            for ni in range(1, NT):
                wsteps += [(ni, ki) for ki in range(KT)]
        wq = []
        widx = [0]

        def emit_next_w():
            if widx[0] < len(wsteps):
                ni, ki = wsteps[widx[0]]
                widx[0] += 1
                nsl = slice(ni * n_tile, (ni + 1) * n_tile)
                ws = wstage.tile([P, n_tile], f32, name="ws")
                nc.sync.dma_start(ws[:], W[ki * P:(ki + 1) * P, nsl])
                wr = wrpool.tile([P, n_tile], f32r, name="wr")
                nc.vector.tensor_copy(wr[:], ws[:])
                wq.append(wr)

        emit_next_w()

        out_dmas = []
        pend = [None]

        def stage_out(ps_tile, mt, nsl_):
            # PSUM -> SBUF, paired: two adjacent m-tiles share one
            # staging tile and one DRAM store (halves the Act SEQ
            # DMA-prep serialization at boundaries). First half on
            # DVE, second on Act so the drain burst fits both queues.
            if pend[0] is None:
                ot2 = otp.tile([P, 2, n_tile], f32, name="ot")
                pend[0] = (ot2, mt)
                nc.vector.tensor_copy(ot2[:, 0, :], ps_tile[:])
            else:
                ot2, lo = pend[0]
                pend[0] = None
                assert mt == lo + 1
                nc.scalar.copy(ot2[:, 1, :], ps_tile[:])
                while out_dmas:
                    emit_out_dma(*out_dmas.pop(0))
                out_dmas.append((lo, nsl_, ot2))

        odma_flip = [0]

        def emit_out_dma(lo, nsl_, ot2):
            # alternate queues so back-to-back stores transfer in parallel
            eng = nc.scalar if odma_flip[0] % 2 == 0 else nc.sync
            odma_flip[0] += 1
            eng.dma_start(
                out[lo * P:(lo + 2) * P, nsl_].rearrange(
                    "(j p) f -> p j f", p=P), ot2[:])

        def flush_out_dmas():
            while out_dmas:
                emit_out_dma(*out_dmas.pop(0))

        mt0 = 0
        first = True
        for mb in m_blocks:
            xtb = xtbp.tile([P, KT, mb * P], f32r, name="xtb")
            nhp = mb // 4
            for hp in range(nhp):
                # ---- fused first n-tile pass (ni=0): transpose + GEMM
                mis = [hp * 4 + i for i in range(4)]
                nsl = slice(0, n_tile)
                pss = [ps_main.tile([P, n_tile], f32, name="mm") for _ in mis]
                pp1 = ps_p1t.tile([r, n_tile], f32, name="mm")
                xs_tiles = {}

                def load_group(g, split=False):
                    for i, mi in enumerate(mis):
                        mt = mt0 + mi
                        xst = xsp.tile([P, kg * P], f32, name="xs")
                        # the very first group gates kernel startup: Pool's
                        # SWDGE descriptor gen is ~1us per DMA serial, so
                        # split it across the Pool and Act queues
                        eng = nc.scalar if split and i % 2 else nc.gpsimd
                        eng.dma_start(
                            xst[:],
                            x[mt * P:(mt + 1) * P, g * kg * P:(g + 1) * kg * P])
                        xs_tiles[(mi, g)] = xst

                def transpose_ki(ki):
                    g, lk = ki // kg, ki % kg
                    for mi in mis:
                        pst = ps_aux.tile([P, P], f32, name="mm")
                        nc.tensor.transpose(
                            pst[:],
                            xs_tiles[(mi, g)][:, lk * P:(lk + 1) * P],
                            ident[:])
                        # copyback rounds to f32r for the main matmuls
                        nc.vector.tensor_copy(
                            xtb[:, ki, mi * P:(mi + 1) * P], pst[:])

                def drain(j, mi):
                    # lora+bias accumulation, then drain
                    mt = mt0 + mi
                    nc.tensor.matmul(
                        pss[j][:], p1t[:, mt * P:(mt + 1) * P], b17[:, nsl],
                        start=False, stop=True)
                    stage_out(pss[j], mt, nsl)

                def pp1_mm(ki):
                    nc.tensor.matmul(
                        pp1[:], a_r[:, ki, :],
                        xtb[:, ki, hp * n_tile:(hp + 1) * n_tile],
                        start=(ki == 0), stop=(ki == KT - 1))

                load_group(0, split=first)
                transpose_ki(0)
                for ki in range(KT):
                    wr = wq.pop(0)
                    emit_next_w()
                    if ki % kg == 0 and ki + kg < KT:
                        load_group(ki // kg + 1)
                    if ki + 1 < KT:
                        transpose_ki(ki + 1)
                    last = ki == KT - 1
                    for j, mi in enumerate(mis):
                        nc.tensor.matmul(
                            pss[j][:], xtb[:, ki, mi * P:(mi + 1) * P],
                            wr[:], start=(ki == 0), stop=False)
                        if last:
                            drain(j, mi)
                    if first and ki == 0:
                        # a_r rounding copy off the startup critical path
                        # (first needed by pp1_mm(0) below)
                        nc.vector.tensor_copy(a_r[:], a_st[:])
                    # pp1 runs one k-tile behind the transposes but one AHEAD
                    # of this loop for ki>=1, so its stop lands in iteration
                    # KT-2 and the p1t rounding copy hides under the last
                    # main matmuls.
                    if ki == 0:
                        pp1_mm(0)
                        pp1_mm(1)
                    elif ki < KT - 1:
                        pp1_mm(ki + 1)
                        if ki == KT - 2:
                            nc.vector.tensor_copy(
                                p1t[0:r, (mt0 + hp * 4) * P:
                                    (mt0 + hp * 4) * P + n_tile],
                                pp1[:])
                    if first and 4 <= ki < 28 and ki % 4 == 0:
                        stage_consts(ki // 4 - 1)
                flush_out_dmas()
                first = False

            # ---- remaining n-tiles: plain GEMM with 8-wide PSUM
            for ni in range(1, NT):
                nsl = slice(ni * n_tile, (ni + 1) * n_tile)
                pss = []
                for mi in range(mb):
                    pool = (ps_main if mi < 4 else
                            ps_aux if mi < 7 else ps_p1t)
                    pss.append(pool.tile([P, n_tile], f32, name="mm"))
                # at the last n-tile of a block, drain the aux/p1t banks
                # first: the next block's transposes/pp1 need those slots
                # before the main banks
                mi_order = (list(range(4, mb)) + list(range(4))
                            if ni == NT - 1 else list(range(mb)))
                for ki in range(KT):
                    wr = wq.pop(0)
                    emit_next_w()
                    last = ki == KT - 1
                    for mi in (mi_order if last else range(mb)):
                        mt = mt0 + mi
                        nc.tensor.matmul(
                            pss[mi][:], xtb[:, ki, mi * P:(mi + 1) * P],
                            wr[:], start=(ki == 0), stop=False)
                        if last:
                            nc.tensor.matmul(
                                pss[mi][:], p1t[:, mt * P:(mt + 1) * P],
                                b17[:, nsl], start=False, stop=True)
                            stage_out(pss[mi], mt, nsl)
                flush_out_dmas()
            mt0 += mb
    nc.compile()
    return nc


_CACHE = {}


def _get_nc(key, *args, **kw):
    if key not in _CACHE:
        _CACHE[key] = build_nc(*args, **kw)
    return _CACHE[key]


def kernel(x, W, bias, lora_A, lora_B, _trace=False):
    Bb, S, D = x.shape
    R = lora_A.shape[1]
    M = Bb * S
    m_core = M // NCORES
    m_blocks = [8, 8]
    nc = _get_nc(("full", m_core, D, R), m_core, D, R, m_blocks)

    xf = np.ascontiguousarray(x.reshape(M, D), dtype=np.float32)
    W = np.ascontiguousarray(W, dtype=np.float32)
    bias = np.ascontiguousarray(bias, dtype=np.float32)
    lora_A = np.ascontiguousarray(lora_A, dtype=np.float32)
    lora_B = np.ascontiguousarray(lora_B, dtype=np.float32)

    in_maps = []
    for c in range(NCORES):
        in_maps.append({
            "x": xf[c * m_core:(c + 1) * m_core],
            "W": W, "bias": bias, "lora_A": lora_A, "lora_B": lora_B,
        })
    res = run_bass_kernel_spmd(nc, in_maps, list(range(NCORES)), trace=_trace)
    outs = [res.results[c]["out"] for c in range(NCORES)]
    full = np.concatenate(outs, axis=0).reshape(Bb, S, D).astype(x.dtype)
    if _trace:
        return full, res
    return full
